# revision 1
# baseline (speedup 1.0000x reference)
"""Trainium2 Bass kernel for fused QKV linear + multi-adapter LoRA (moe_routing).

Reference computation (all fp32):
    base = x @ W^T + bias                      x:[B,S,D]  W:[3D,D]
    tmp[p,n,b,s,r]  = x . lora_A[p,n,r,:]      (down-projection, rank 16)
    tmp *= scaling[n] * lora_masks[n,b]
    lora[p,b,s,o]   = tmp . lora_B[p,n,o,r]    (up-projection, summed over n)
    out = base + concat_p(lora)                [B,S,3D]

Sharding: row-parallel over the flattened (B*S) dimension — each of the 8
cores computes 1024 rows x all 12288 output columns.  This avoids
replicating the LoRA down-projection (which column-parallel sharding would
require) and makes the per-batch adapter mask a single per-core [128]
vector.  Each core holds x^T for its rows resident in SBUF and streams W.

Device layout (per core, all matmuls bf16 with fp32 PSUM accumulation):
    xk  [128, 32, 1024]  x^T tiles: [k%128, k//128, m]
    wk  [96, 128, 32, 128] W^T tiles per output tile: [ot, k%128, k//128, o]
    at  [128, 3, 32, 128]  lora_A^T tiles: [k%128, p, k//128, nr]
    bt  [3, 128, 4096]     lora_B^T: [p, nr, o]   (nr = n*16 + r)
    bias[128, 96]          bias[ot*128+op] at [op, ot]
    wv  [128, 1]           scaling[n]*mask[n, batch(core)] at [n*16+r]
    out [96, 128, 1024]    out^T tiles: [ot, o, m]

Per output tile ot (96 of them): 32 k-tile matmuls accumulate
W^T x into PSUM [o=128, m=1024], then one extra matmul per 512-wide m
chunk accumulates the LoRA up-projection (contraction over nr=128) into
the same PSUM group, then a DVE tensor_scalar add applies bias while
copying PSUM -> SBUF, then DMA out.
"""

import numpy as np
import ml_dtypes
from contextlib import ExitStack

import concourse.bass as bass
import concourse.tile as tile
from concourse import bacc, mybir
from concourse.bass_utils import run_bass_kernel_spmd

BF16 = ml_dtypes.bfloat16

B, S, D = 4, 2048, 4096
OUT = 3 * D
N_CORES = 8
M = B * S                 # 8192 flattened rows
MC = M // N_CORES         # 1024 rows per core
P = 128
KT = D // P               # 32 k-tiles
OT = OUT // P             # 96 output tiles
OTP = OT // 3             # 32 output tiles per q/k/v block
NADP, R = 8, 16
NR = NADP * R             # 128 = contraction size of the up-projection
MM_N = 512                # moving-operand width per matmul
N_MCHUNK = MC // MM_N     # 2

_CACHE: dict = {}


def _build():
    """Trace + compile the per-core Bass program (same program on all cores)."""
    fp32 = mybir.dt.float32
    bf16 = mybir.dt.bfloat16

    nc = bacc.Bacc("TRN2", target_bir_lowering=False, debug=False,
                   num_devices=N_CORES)
    xk = nc.dram_tensor("xk", [P, KT, MC], bf16, kind="ExternalInput").ap()
    wk = nc.dram_tensor("wk", [OT, P, KT, P], bf16, kind="ExternalInput").ap()
    at = nc.dram_tensor("at", [P, 3, KT, NR], bf16, kind="ExternalInput").ap()
    bt = nc.dram_tensor("bt", [3, NR, D], bf16, kind="ExternalInput").ap()
    bias = nc.dram_tensor("bias", [P, OT], fp32, kind="ExternalInput").ap()
    wv = nc.dram_tensor("wv", [P, 1], fp32, kind="ExternalInput").ap()
    out = nc.dram_tensor("out", [OT, P, MC], fp32, kind="ExternalOutput").ap()

    with tile.TileContext(nc) as tc, ExitStack() as ctx:
        const = ctx.enter_context(tc.tile_pool(name="const", bufs=1))
        wpool = ctx.enter_context(tc.tile_pool(name="wpool", bufs=4))
        btpool = ctx.enter_context(tc.tile_pool(name="btpool", bufs=2))
        opool = ctx.enter_context(tc.tile_pool(name="opool", bufs=4))
        dppool = ctx.enter_context(tc.tile_pool(name="dppool", bufs=2, space="PSUM"))
        pspool = ctx.enter_context(tc.tile_pool(name="pspool", bufs=2, space="PSUM"))

        # Resident inputs.  x is split per k-tile so the loads spread across
        # DMA queues instead of serializing on one engine.
        xsb = const.tile([P, KT, MC], bf16)
        for kt in range(KT):
            nc.sync.dma_start(xsb[:, kt, :], xk[:, kt, :])
        asb = const.tile([P, 3, KT, NR], bf16)
        nc.sync.dma_start(asb, at)
        biassb = const.tile([P, OT], fp32)
        nc.sync.dma_start(biassb, bias)
        wvsb = const.tile([P, 1], fp32)
        nc.sync.dma_start(wvsb, wv)
        # Scaled down-projection result (x A^T * wv)^T, bf16: [nr, p, m]
        tmpsb = const.tile([P, 3, MC], bf16)

        # LoRA down-projection: tmp^T[nr, m] = A_p^T.T @ x^T, per p and m-chunk.
        for p in range(3):
            for mc_i in range(N_MCHUNK):
                msl = slice(mc_i * MM_N, (mc_i + 1) * MM_N)
                dp = dppool.tile([P, MM_N], fp32)
                for kt in range(KT):
                    nc.tensor.matmul(dp, lhsT=asb[:, p, kt, :],
                                     rhs=xsb[:, kt, msl],
                                     start=(kt == 0), stop=(kt == KT - 1))
                # scale by per-partition adapter weight while copying to SBUF
                nc.scalar.mul(tmpsb[:, p, msl], dp, wvsb)

        # Main loop: 96 output tiles of [o=128, m=1024].
        for p in range(3):
            btsb = btpool.tile([NR, D], bf16)
            for j in range(4):
                osl = slice(j * (D // 4), (j + 1) * (D // 4))
                nc.sync.dma_start(btsb[:, osl], bt[p, :, osl])
            for j in range(OTP):
                ot = p * OTP + j
                wsb = wpool.tile([P, KT, P], bf16)
                for h in range(4):
                    ksl = slice(h * (KT // 4), (h + 1) * (KT // 4))
                    nc.sync.dma_start(wsb[:, ksl, :], wk[ot, :, ksl, :])
                ps = pspool.tile([P, MC], fp32)
                for kt in range(KT):
                    for mc_i in range(N_MCHUNK):
                        msl = slice(mc_i * MM_N, (mc_i + 1) * MM_N)
                        nc.tensor.matmul(ps[:, msl], lhsT=wsb[:, kt, :],
                                         rhs=xsb[:, kt, msl],
                                         start=(kt == 0), stop=False)
                for mc_i in range(N_MCHUNK):
                    msl = slice(mc_i * MM_N, (mc_i + 1) * MM_N)
                    nc.tensor.matmul(ps[:, msl],
                                     lhsT=btsb[:, j * P:(j + 1) * P],
                                     rhs=tmpsb[:, p, msl],
                                     start=False, stop=True)
                osb = opool.tile([P, MC], fp32)
                nc.vector.tensor_scalar_add(osb, ps, biassb[:, ot:ot + 1])
                nc.sync.dma_start(out[ot], osb)

    nc.compile()
    return nc


def get_nc():
    if "nc" not in _CACHE:
        _CACHE["nc"] = _build()
    return _CACHE["nc"]


def prep_in_maps(inputs: dict) -> list[dict]:
    """Shard + retile the full inputs into the 8 per-core input maps."""
    x = np.asarray(inputs["x"], np.float32).reshape(M, D)
    w = np.asarray(inputs["weight"], np.float32)
    bias = np.asarray(inputs["bias"], np.float32)
    lora_A = np.asarray(inputs["lora_A"], np.float32)
    lora_B = np.asarray(inputs["lora_B"], np.float32)
    scaling = np.asarray(inputs["scaling"], np.float32)
    masks = np.asarray(inputs["lora_masks"], np.float32)

    wk = np.ascontiguousarray(
        w.reshape(OT, P, KT, P).transpose(0, 3, 2, 1)).astype(BF16)
    at = np.ascontiguousarray(
        lora_A.reshape(3, NR, KT, P).transpose(3, 0, 2, 1)).astype(BF16)
    bt = np.ascontiguousarray(
        lora_B.transpose(0, 1, 3, 2).reshape(3, NR, D)).astype(BF16)
    biasd = np.ascontiguousarray(bias.reshape(OT, P).T)
    wmat = scaling[:, None] * masks          # [n, b]

    in_maps = []
    for c in range(N_CORES):
        xs = x[c * MC:(c + 1) * MC]          # [MC, D]
        xkc = np.ascontiguousarray(
            xs.reshape(MC, KT, P).transpose(2, 1, 0)).astype(BF16)
        b_idx = (c * MC) // S                # batch of this core's rows
        wvc = np.repeat(wmat[:, b_idx], R).astype(np.float32).reshape(P, 1)
        in_maps.append({"xk": xkc, "wk": wk, "at": at, "bt": bt,
                        "bias": biasd, "wv": wvc})
    return in_maps


def run_device(in_maps: list[dict]):
    nc = get_nc()
    return run_bass_kernel_spmd(nc, in_maps, core_ids=list(range(N_CORES)))


def assemble(results: list[dict]) -> np.ndarray:
    big = np.empty((M, OUT), np.float32)
    for c in range(N_CORES):
        big[c * MC:(c + 1) * MC] = results[c]["out"].reshape(OUT, MC).T
    return big.reshape(B, S, OUT)


def kernel(**inputs) -> np.ndarray:
    in_maps = prep_in_maps(inputs)
    res = run_device(in_maps)
    return assemble(res.results)


# revision 2
# speedup vs baseline: 12.5288x; 12.5288x over previous
"""Trainium2 Bass kernel for fused QKV linear + multi-adapter LoRA (moe_routing).

Reference computation (all fp32):
    base = x @ W^T + bias                      x:[B,S,D]  W:[3D,D]
    tmp[p,n,b,s,r]  = x . lora_A[p,n,r,:]      (down-projection, rank 16)
    tmp *= scaling[n] * lora_masks[n,b]
    lora[p,b,s,o]   = tmp . lora_B[p,n,o,r]    (up-projection, summed over n)
    out = base + concat_p(lora)                [B,S,3D]

Sharding: row-parallel over the flattened (B*S) dimension — each of the 8
cores computes 1024 rows x all 12288 output columns.  Unlike the
column-parallel split this does not replicate the LoRA down-projection
(which is ~25% of the base GEMM's FLOPs), and the per-batch adapter mask
becomes a single per-core [128] vector (each core's rows live in one
batch).  Each core holds x^T for its rows resident in SBUF and streams W.

Device layout (per core, all matmuls bf16 with fp32 PSUM accumulation):
    xk  [128, 32, 1024]    x^T tiles: [k%128, k//128, m]
    wk  [96, 128, 32, 128] W^T tiles per output tile: [ot, k%128, k//128, o]
    at  [128, 3, 32, 128]  lora_A^T tiles: [k%128, p, k//128, nr]
    bt  [3, 128, 4096]     lora_B^T: [p, nr, o]   (nr = n*16 + r)
    bias[128, 96]          bias[ot*128+op] at [op, ot]
    wv  [128, 1]           scaling[n]*mask[n, batch(core)] at [n*16+r]
    out [96, 128, 1024]    out^T tiles: [ot, o, m]

Per output tile ot (96): 32 k-tile matmuls accumulate W^T x into PSUM
[o=128, m=1024]; one extra matmul per 512-wide m chunk accumulates the
LoRA up-projection (contraction over nr=128) into the same PSUM group; a
DVE tensor_scalar add applies bias while copying PSUM -> SBUF; DMA out.

Measured (8x axon trn2, effective PE clock ~1.86 GHz): ~1.76 ms/core,
which matches the PE roofline for the 6528 N=512 matmuls this schedule
issues (~275 ns each).  bf16 end-to-end relative error vs the fp32
reference: ~2.0e-3.
"""

import numpy as np
import ml_dtypes
from contextlib import ExitStack

import concourse.bass as bass
import concourse.tile as tile
from concourse import bacc, mybir
from concourse.bass_utils import run_bass_kernel_spmd

BF16 = ml_dtypes.bfloat16

B, S, D = 4, 2048, 4096
OUT = 3 * D
N_CORES = 8
M = B * S                 # 8192 flattened rows
MC = M // N_CORES         # 1024 rows per core
P = 128
KT = D // P               # 32 k-tiles
OT = OUT // P             # 96 output tiles
OTP = OT // 3             # 32 output tiles per q/k/v block
NADP, R = 8, 16
NR = NADP * R             # 128 = contraction size of the up-projection
MM_N = 512                # moving-operand width per matmul
N_MCHUNK = MC // MM_N     # 2

_CACHE: dict = {}


def _build(loop_iters: int | None = None):
    """Trace + compile the per-core Bass program (same program on all cores).

    loop_iters: if set, wrap the body in a hardware For loop that executes
    it that many times per dispatch (used only for benchmarking)."""
    fp32 = mybir.dt.float32
    bf16 = mybir.dt.bfloat16

    nc = bacc.Bacc("TRN2", target_bir_lowering=False, debug=False,
                   num_devices=N_CORES)
    xk = nc.dram_tensor("xk", [P, KT, MC], bf16, kind="ExternalInput").ap()
    wk = nc.dram_tensor("wk", [OT, P, KT, P], bf16, kind="ExternalInput").ap()
    at = nc.dram_tensor("at", [P, 3, KT, NR], bf16, kind="ExternalInput").ap()
    bt = nc.dram_tensor("bt", [3, NR, D], bf16, kind="ExternalInput").ap()
    bias = nc.dram_tensor("bias", [P, OT], fp32, kind="ExternalInput").ap()
    wv = nc.dram_tensor("wv", [P, 1], fp32, kind="ExternalInput").ap()
    out = nc.dram_tensor("out", [OT, P, MC], fp32, kind="ExternalOutput").ap()

    with tile.TileContext(nc) as tc, ExitStack() as ctx:
        const = ctx.enter_context(tc.tile_pool(name="const", bufs=1))
        wpool = ctx.enter_context(tc.tile_pool(name="wpool", bufs=6))
        btpool = ctx.enter_context(tc.tile_pool(name="btpool", bufs=2))
        opool = ctx.enter_context(tc.tile_pool(name="opool", bufs=4))
        dppool = ctx.enter_context(tc.tile_pool(name="dppool", bufs=2, space="PSUM"))
        pspool = ctx.enter_context(tc.tile_pool(name="pspool", bufs=2, space="PSUM"))

        loop_cm = tc.For_i(0, loop_iters, 1) if loop_iters else None
        if loop_cm is not None:
            loop_cm.__enter__()
        try:
            # Resident inputs.  x is split per k-tile so the loads spread
            # across DMA queues; gpsimd (SWDGE) keeps the sync HWDGE ring
            # free for the W stream.
            xsb = const.tile([P, KT, MC], bf16, name="xsb")
            for kt in range(KT):
                nc.gpsimd.dma_start(xsb[:, kt, :], xk[:, kt, :])
            asb = const.tile([P, 3, KT, NR], bf16, name="asb")
            nc.gpsimd.dma_start(asb, at)
            biassb = const.tile([P, OT], fp32, name="biassb")
            nc.gpsimd.dma_start(biassb, bias)
            wvsb = const.tile([P, 1], fp32, name="wvsb")
            nc.gpsimd.dma_start(wvsb, wv)
            # Scaled down-projection result (x A^T * wv)^T, bf16: [nr, p, m]
            tmpsb = const.tile([P, 3, MC], bf16, name="tmpsb")

            # LoRA down-projection: tmp^T[nr, m] = A_p^T.T @ x^T per p/chunk.
            for p in range(3):
                for mc_i in range(N_MCHUNK):
                    msl = slice(mc_i * MM_N, (mc_i + 1) * MM_N)
                    dp = dppool.tile([P, MM_N], fp32, name="dp")
                    for kt in range(KT):
                        nc.tensor.matmul(dp, lhsT=asb[:, p, kt, :],
                                         rhs=xsb[:, kt, msl],
                                         start=(kt == 0), stop=(kt == KT - 1))
                    # scale by the per-partition adapter weight while
                    # copying PSUM -> SBUF
                    nc.scalar.mul(tmpsb[:, p, msl], dp, wvsb)

            # Main loop: 96 output tiles of [o=128, m=1024].
            for p in range(3):
                btsb = btpool.tile([NR, D], bf16, name="btsb")
                for jj in range(4):
                    osl = slice(jj * (D // 4), (jj + 1) * (D // 4))
                    nc.gpsimd.dma_start(btsb[:, osl], bt[p, :, osl])
                for j in range(OTP):
                    ot = p * OTP + j
                    wsb = wpool.tile([P, KT, P], bf16, name="wsb")
                    for h in range(4):
                        ksl = slice(h * (KT // 4), (h + 1) * (KT // 4))
                        nc.sync.dma_start(wsb[:, ksl, :], wk[ot, :, ksl, :])
                    ps = pspool.tile([P, MC], fp32, name="ps")
                    for kt in range(KT):
                        for mc_i in range(N_MCHUNK):
                            msl = slice(mc_i * MM_N, (mc_i + 1) * MM_N)
                            nc.tensor.matmul(ps[:, msl], lhsT=wsb[:, kt, :],
                                             rhs=xsb[:, kt, msl],
                                             start=(kt == 0), stop=False)
                    for mc_i in range(N_MCHUNK):
                        msl = slice(mc_i * MM_N, (mc_i + 1) * MM_N)
                        nc.tensor.matmul(ps[:, msl],
                                         lhsT=btsb[:, j * P:(j + 1) * P],
                                         rhs=tmpsb[:, p, msl],
                                         start=False, stop=True)
                    osb = opool.tile([P, MC], fp32, name="osb")
                    nc.vector.tensor_scalar_add(osb, ps, biassb[:, ot:ot + 1])
                    nc.scalar.dma_start(out[ot], osb)
        finally:
            if loop_cm is not None:
                loop_cm.__exit__(None, None, None)

    nc.compile()
    return nc


def get_nc(loop_iters: int | None = None):
    key = ("nc", loop_iters)
    if key not in _CACHE:
        _CACHE[key] = _build(loop_iters)
    return _CACHE[key]


def prep_in_maps(inputs: dict) -> list[dict]:
    """Shard + retile the full inputs into the 8 per-core input maps."""
    x = np.asarray(inputs["x"], np.float32).reshape(M, D)
    w = np.asarray(inputs["weight"], np.float32)
    bias = np.asarray(inputs["bias"], np.float32)
    lora_A = np.asarray(inputs["lora_A"], np.float32)
    lora_B = np.asarray(inputs["lora_B"], np.float32)
    scaling = np.asarray(inputs["scaling"], np.float32)
    masks = np.asarray(inputs["lora_masks"], np.float32)

    wk = np.ascontiguousarray(
        w.reshape(OT, P, KT, P).transpose(0, 3, 2, 1)).astype(BF16)
    at = np.ascontiguousarray(
        lora_A.reshape(3, NR, KT, P).transpose(3, 0, 2, 1)).astype(BF16)
    bt = np.ascontiguousarray(
        lora_B.transpose(0, 1, 3, 2).reshape(3, NR, D)).astype(BF16)
    biasd = np.ascontiguousarray(bias.reshape(OT, P).T)
    wmat = scaling[:, None] * masks          # [n, b]

    in_maps = []
    for c in range(N_CORES):
        xs = x[c * MC:(c + 1) * MC]          # [MC, D]
        xkc = np.ascontiguousarray(
            xs.reshape(MC, KT, P).transpose(2, 1, 0)).astype(BF16)
        b_idx = (c * MC) // S                # batch of this core's rows
        wvc = np.repeat(wmat[:, b_idx], R).astype(np.float32).reshape(P, 1)
        in_maps.append({"xk": xkc, "wk": wk, "at": at, "bt": bt,
                        "bias": biasd, "wv": wvc})
    return in_maps


def run_device(in_maps: list[dict]):
    nc = get_nc()
    return run_bass_kernel_spmd(nc, in_maps, core_ids=list(range(N_CORES)))


def assemble(results: list[dict]) -> np.ndarray:
    big = np.empty((M, OUT), np.float32)
    for c in range(N_CORES):
        big[c * MC:(c + 1) * MC] = results[c]["out"].reshape(OUT, MC).T
    return big.reshape(B, S, OUT)


def kernel(**inputs) -> np.ndarray:
    in_maps = prep_in_maps(inputs)
    res = run_device(in_maps)
    return assemble(res.results)
